# revision 7
# baseline (speedup 1.0000x reference)
"""KnowledgeAwareAttention Trainium2 kernel (8-core SPMD, row-sharded).

attn[i,j] = sum_d R_emb[q[i,j],d] * x[j,d] * x[i,d]
out = softmax(attn, -1) @ x

Strategy per core (128 output rows):
  - PE computes 42 relation "planes" T_k = (x_I * R_k) @ x^T  [128,1024]
    (contraction over d=256 in two 128-chunks, PSUM-accumulated).
  - The per-element selection attn[i,j] = T_{q[i,j]}[i,j] is a binary mux
    tree over the 6 bits of q: 21 ScalarE PSUM->SBUF copies (even planes)
    + 41 VectorE copy_predicated merges with host-precomputed bit masks.
  - softmax without max-subtraction (scores are tiny: |attn| < ~0.2),
    exp on ScalarE with fused row-sum (accum_out), reciprocal on VectorE.
  - output matmul: 8 PE transposes of the exp-plane + 8 accumulating
    matmuls against x chunks; final row-scale by 1/Z fused into the
    PSUM->SBUF copy on ScalarE.
Inputs are sharded/prepared on host: q bit-planes as f32 masks, x^T,
x^T block columns, R^T (all f32).
"""

import numpy as np

import concourse.bass as bass
import concourse.mybir as mybir
import concourse.tile as tile
from concourse.bass_utils import run_bass_kernel_spmd
from concourse.masks import make_identity

B = 1024
D = 256
NREL = 42
NCORES = 8
P = 128  # rows per core
F32 = mybir.dt.float32
AF = mybir.ActivationFunctionType


def _patch_tile_tail_drain():
    """This container's walrus rejects >1 sync-wait command on the
    kernel-tail SP Drain. Split the waits across SP nops."""
    import concourse.mybir as mybir_
    import concourse.tile as tile_

    def _drain_and_barrier(self, tick_clock, wait_clock):
        nc = self.nc
        drain_inst = nc.sync.drain()
        wait_clock.add_sem_waits(
            drain_inst.ins, tile_.ScopedClock({None: tick_clock.global_clock})
        )
        si = drain_inst.ins.sync_info
        waits = list(si.on_wait) if si and si.on_wait else []
        if len(waits) > 1:
            si.on_wait = waits[:1]
            for w in waits[1:]:
                nop = nc.sync.nop(nofuse=True)
                nop.ins.sync_info = mybir_.SyncInfo(on_wait=[w], on_update=[])
        nc.all_engine_barrier()
        assert self.sems is not None
        popped = nc._tile_sem_poison_stack.pop()
        assert popped is self._sem_poison
        nc.clear_and_free_semaphores(list(self.sems.allocated().values()))
        nc.all_engine_barrier()

    tile_.TileContext._drain_and_barrier = _drain_and_barrier


_patch_tile_tail_drain()


_MAX_WAITS = 1


def _split_excess_waits(nc: bass.Bass, max_waits: int = _MAX_WAITS) -> None:
    """This container's walrus caps the number of sync-wait commands one
    instruction may carry. Move excess waits onto same-engine NoOps
    inserted immediately before the instruction."""
    cnt = 0
    for wrapper in nc.bb_map.values():
        bb = wrapper.bb
        old = list(bb.instructions)
        new = []
        changed = False
        for ins in old:
            si = ins.sync_info
            waits = list(si.on_wait) if si and si.on_wait else []
            if len(waits) > max_waits:
                changed = True
                si.on_wait = waits[:max_waits]
                rest = waits[max_waits:]
                for i in range(0, len(rest), max_waits):
                    nop = mybir.InstNoOp(name=f"waitnop{cnt}", ins=[], outs=[])
                    cnt += 1
                    nop.engine = ins.engine
                    nop.sync_info = mybir.SyncInfo(
                        on_wait=rest[i:i + max_waits], on_update=[]
                    )
                    new.append(nop)
            new.append(ins)
        if changed:
            bb.instructions = new


def build_nc() -> bass.Bass:
    nc = bass.Bass()
    xT_d = nc.dram_tensor("xt", [D, B], F32, kind="ExternalInput")
    xTI_d = nc.dram_tensor("xti", [D, P], F32, kind="ExternalInput")
    x_d = nc.dram_tensor("x", [B, D], F32, kind="ExternalInput")
    rt_d = nc.dram_tensor("rt", [D, NREL], F32, kind="ExternalInput")
    bits_d = nc.dram_tensor("bits", [6 * P, B], mybir.dt.int8, kind="ExternalInput")
    out_d = nc.dram_tensor("out", [P, D], F32, kind="ExternalOutput")

    with tile.TileContext(nc) as tc:
        with (
            tc.tile_pool(name="const", bufs=1) as const,
            tc.tile_pool(name="lhs", bufs=4) as lhsp,
            tc.tile_pool(name="planes", bufs=1) as planep,
            tc.tile_pool(name="sm", bufs=1) as smp,
            tc.tile_pool(name="et", bufs=4) as etp,
        ):
            # ---- loads ----
            xT = [const.tile([P, B], F32, tag=f"xt{c}", name=f"xt{c}") for c in range(2)]
            xTI = [const.tile([P, P], F32, tag=f"xti{c}", name=f"xti{c}") for c in range(2)]
            rt = [const.tile([P, NREL], F32, tag=f"rt{c}", name=f"rt{c}") for c in range(2)]
            for c in range(2):
                nc.sync.dma_start(xT[c][:, :], xT_d[c * P:(c + 1) * P, :])
                nc.sync.dma_start(xTI[c][:, :], xTI_d[c * P:(c + 1) * P, :])
                nc.sync.dma_start(rt[c][:, :], rt_d[c * P:(c + 1) * P, :])
            bits = [const.tile([P, B], mybir.dt.int8, tag=f"b{l}", name=f"b{l}") for l in range(6)]
            for l in range(6):
                nc.sync.dma_start(bits[l][:, :], bits_d[l * P:(l + 1) * P, :])
            xc = [const.tile([P, D], F32, tag=f"x{j}", name=f"x{j}") for j in range(8)]
            for j in range(8):
                nc.sync.dma_start(xc[j][:, :], x_d[j * P:(j + 1) * P, :])
            ident = const.tile([P, P], F32, tag="ident")
            make_identity(nc, ident[:, :])

            # ---- phase B: 42 planes + tree level 0 ----
            planes = []
            with tc.tile_pool(name="pp", bufs=4, space="PSUM") as pp:
                for m in range(21):
                    P_m = None
                    for parity in (0, 1):
                        k = 2 * m + parity
                        pt = pp.tile([P, B], F32, tag="plane", name=f"t{k}")
                        for c in range(2):
                            lh = lhsp.tile([P, P], F32, tag="lh", name=f"lh{k}_{c}")
                            nc.vector.tensor_scalar_mul(
                                lh[:, :], xTI[c][:, :], rt[c][:, k:k + 1]
                            )
                            for jh in range(2):
                                nc.tensor.matmul(
                                    pt[:, jh * 512:(jh + 1) * 512],
                                    lhsT=lh[:, :],
                                    rhs=xT[c][:, jh * 512:(jh + 1) * 512],
                                    start=(c == 0),
                                    stop=(c == 1),
                                )
                        if parity == 0:
                            P_m = planep.tile([P, B], F32, tag=f"p{m}", name=f"p{m}")
                            nc.scalar.copy(P_m[:, :], pt[:, :])
                        else:
                            nc.vector.copy_predicated(
                                P_m[:, :], bits[0][:, :], pt[:, :]
                            )
                    planes.append(P_m)

            # ---- phase C: tree levels 1..5 ----
            lvl_planes = planes
            for lvl in range(1, 6):
                nxt = []
                for m in range(len(lvl_planes) // 2):
                    a, b = lvl_planes[2 * m], lvl_planes[2 * m + 1]
                    nc.vector.copy_predicated(a[:, :], bits[lvl][:, :], b[:, :])
                    nxt.append(a)
                if len(lvl_planes) % 2:
                    nxt.append(lvl_planes[-1])
                lvl_planes = nxt
            attn = lvl_planes[0]

            # ---- phase D: exp + row sums ----
            E = smp.tile([P, B], F32, tag="E")
            z = smp.tile([P, 1], F32, tag="z")
            rz = smp.tile([P, 1], F32, tag="rz")
            nc.scalar.activation(E[:, :], attn[:, :], AF.Exp, accum_out=z[:, :])
            nc.vector.reciprocal(rz[:, :], z[:, :])

            # ---- phase E: transposes + output matmul ----
            with (
                tc.tile_pool(name="tp", bufs=2, space="PSUM") as tp,
                tc.tile_pool(name="op", bufs=1, space="PSUM") as op,
            ):
                out_ps = op.tile([P, D], F32, tag="out")
                for jc in range(8):
                    ptile = tp.tile([P, P], F32, tag="tp", name=f"tp{jc}")
                    nc.tensor.transpose(ptile[:, :], E[:, jc * P:(jc + 1) * P], ident[:, :])
                    et = etp.tile([P, P], F32, tag="et", name=f"et{jc}")
                    nc.scalar.copy(et[:, :], ptile[:, :])
                    nc.tensor.matmul(
                        out_ps[:, :],
                        lhsT=et[:, :],
                        rhs=xc[jc][:, :],
                        start=(jc == 0),
                        stop=(jc == 7),
                    )
                # ---- phase F: scale rows by 1/Z and store ----
                out_sb = smp.tile([P, D], F32, tag="osb")
                nc.scalar.activation(out_sb[:, :], out_ps[:, :], AF.Copy, scale=rz[:, :])
                nc.sync.dma_start(out_d[:, :], out_sb[:, :])
    _split_excess_waits(nc)
    return nc


_NC_CACHE = None


def _get_nc():
    global _NC_CACHE
    if _NC_CACHE is None:
        _NC_CACHE = build_nc()
    return _NC_CACHE


def kernel(x, x_mask, q, f, R_emb):
    x = np.asarray(x, dtype=np.float32)
    q = np.asarray(q)
    R = np.asarray(R_emb, dtype=np.float32)

    xT = np.ascontiguousarray(x.T)                      # [D, B]
    rt = np.ascontiguousarray(R.T)                      # [D, 42]
    q32 = q.astype(np.int32)

    in_maps = []
    for c in range(NCORES):
        rows = slice(c * P, (c + 1) * P)
        qb = q32[rows]                                   # [128, B]
        bits = np.empty((6 * P, B), dtype=np.int8)
        for l in range(6):
            bits[l * P:(l + 1) * P] = ((qb >> l) & 1).astype(np.int8)
        in_maps.append(
            {
                "xt": xT,
                "xti": np.ascontiguousarray(xT[:, rows]),
                "x": x,
                "rt": rt,
                "bits": bits,
            }
        )

    res = run_bass_kernel_spmd(_get_nc(), in_maps, core_ids=list(range(NCORES)))
    out = np.concatenate([res.results[c]["out"] for c in range(NCORES)], axis=0)
    return out
